# revision 1
# baseline (speedup 1.0000x reference)
"""Trainium2 Bass kernel for nn_Decoder_58377195487266.

Single-layer decoder: shared-head causal attention (d_k=32) + FFN(256->1024->256)
with two LayerNorms. B=16, T=2048, EMB=256.

Sharding: pure data-parallel over batch. 8 cores x 2 batches each, weights
replicated, no collectives.

Host-side algebraic folds (exact):
  - cat = tile(attn, 8)  =>  cat @ Wp == attn @ (sum of Wp's 8 row blocks).
  - 1/sqrt(d_k) score scale folded into Wq.
  - bp folded into the residual input (x + bp).
  - softmax denominator: ones-column appended to v + unit column appended to
    the folded Wp, so the mh matmul also emits sum_s exp(score) per token;
    normalization multiplies by its reciprocal during z1 assembly.
"""

import math
import os

import numpy as np

import concourse.bass as bass  # noqa: F401
import concourse.mybir as mybir
from concourse import bacc
from concourse.bass_utils import run_bass_kernel_spmd
from concourse.masks import make_identity
from concourse.tile import TileContext

F32 = mybir.dt.float32
F32R = mybir.dt.float32r
I32 = mybir.dt.int32
AF = mybir.ActivationFunctionType
OP = mybir.AluOpType

B, T, EMB = 16, 2048, 256
DK = 32
HID = 4 * EMB
N_CORES = 8
B_LOC = B // N_CORES  # 2
NT = T // 128         # 16 t-blocks
NCH = T // 512        # 4 chunks
LN_EPS = 1e-5

RSQRT_MAGIC = 0x5F3759DF


def _nr_rsqrt(nc, pool, out, varp):
    """out = 1/sqrt(varp) via bit-trick + 3 Newton iterations, all on DVE."""
    sh = list(varp.shape)
    yi = pool.tile(sh, I32, tag="nr_i", bufs=2)
    magic = pool.tile(sh, I32, tag="nr_m", bufs=2)
    nc.vector.memset(magic[:], RSQRT_MAGIC)
    nc.vector.tensor_scalar(yi[:], varp.bitcast(I32), 1, None, OP.logical_shift_right)
    nc.vector.tensor_tensor(yi[:], magic[:], yi[:], OP.subtract)
    y = yi.bitcast(F32)
    e = pool.tile(sh, F32, tag="nr_e", bufs=2)
    h = pool.tile(sh, F32, tag="nr_h", bufs=2)
    for _ in range(3):
        nc.vector.tensor_tensor(e[:], y[:], y[:], OP.mult)
        nc.vector.tensor_tensor(e[:], e[:], varp[:], OP.mult)
        nc.vector.tensor_scalar(h[:], e[:], -0.5, 1.5, OP.mult, OP.add)
        nc.vector.tensor_tensor(y[:], y[:], h[:], OP.mult)
    nc.vector.tensor_copy(out[:], y[:])


def _layernorm(nc, st_pool, sm_pool, z_sb, y_sb):
    """Natural-layout LN: stats from accumulated sums, NR rsqrt, gpsimd apply.

    z_sb: [128, NT, EMB] fp32. Writes normalized (no affine) y_sb.
    """
    stats6 = st_pool.tile([128, NT, 6], F32, tag="st6", bufs=2)
    agg = st_pool.tile([128, NT, 2], F32, tag="agg", bufs=2)
    for tb in range(NT):
        nc.vector.bn_stats(stats6[:, tb], z_sb[:, tb])
        nc.vector.bn_aggr(agg[:, tb], stats6[:, tb])
    mean = agg[:, :, 0]
    varp = st_pool.tile([128, NT], F32, tag="varp", bufs=2)
    rstd = st_pool.tile([128, NT], F32, tag="rstd", bufs=2)
    mrstd = st_pool.tile([128, NT], F32, tag="mrstd", bufs=2)
    nc.vector.tensor_scalar(varp[:], agg[:, :, 1], 1.0, LN_EPS, OP.mult, OP.add)
    _nr_rsqrt(nc, st_pool, rstd, varp)
    nc.vector.tensor_tensor(mrstd[:], mean, rstd[:], OP.mult)
    for tb in range(NT):
        eng = nc.vector if os.environ.get("KDBG_NO_GPSIMD") else nc.gpsimd
        eng.tensor_scalar(
            y_sb[:, tb], z_sb[:, tb],
            rstd[:, tb:tb + 1], mrstd[:, tb:tb + 1],
            OP.mult, OP.subtract,
        )


def build_decoder(apply_g1be1: bool, apply_g2be2: bool, apply_b2: bool):
    """Build the per-core Bass program (B_LOC batches, full T each)."""
    PH = int(os.environ.get("KDBG_PHASE", "99"))
    nc = bacc.Bacc(None, target_bir_lowering=False)

    xp_d = nc.dram_tensor("xp", [B_LOC, T, EMB], F32, kind="ExternalInput")
    xt_d = nc.dram_tensor("xt", [B_LOC, EMB, T], F32, kind="ExternalInput")
    wq4_d = nc.dram_tensor("wq4", [EMB, 128], F32, kind="ExternalInput")
    wk4_d = nc.dram_tensor("wk4", [EMB, 128], F32, kind="ExternalInput")
    wv_d = nc.dram_tensor("wv", [EMB, DK], F32, kind="ExternalInput")
    wpf_d = nc.dram_tensor("wpf", [DK + 2, EMB + 2], F32, kind="ExternalInput")
    w1_d = nc.dram_tensor("w1", [EMB, HID], F32, kind="ExternalInput")
    b1_d = nc.dram_tensor("b1", [128, 8], F32, kind="ExternalInput")
    w2_d = nc.dram_tensor("w2", [HID, EMB], F32, kind="ExternalInput")
    aff_d = nc.dram_tensor("aff", [1, 5, EMB], F32, kind="ExternalInput")
    # aff rows: b2, g1, be1, g2, be2
    out_d = nc.dram_tensor("out", [B_LOC, T, EMB], F32, kind="ExternalOutput")

    need_bcast = apply_g1be1 or apply_g2be2

    with TileContext(nc) as tc:
        with (
            tc.tile_pool(name="wpool", bufs=1) as wp,
            tc.tile_pool(name="xpool", bufs=2) as xq,
            tc.tile_pool(name="qkpool", bufs=1) as qk_pool,
            tc.tile_pool(name="atpool", bufs=4) as at_pool,
            tc.tile_pool(name="bigpool", bufs=1) as big_pool,
            tc.tile_pool(name="hpool", bufs=8) as h_pool,
            tc.tile_pool(name="stats", bufs=2) as st_pool,
            tc.tile_pool(name="small", bufs=3) as sm_pool,
        ):
            # ---------- weights / constants ----------
            ident = wp.tile([128, 128], F32)
            make_identity(nc, ident[:])
            wq4_sb = wp.tile([128, 2, 128], F32R)
            nc.sync.dma_start(
                wq4_sb[:], wq4_d.rearrange("(eb p) m -> p eb m", p=128).bitcast(F32R)
            )
            wk4_sb = wp.tile([128, 2, 128], F32R)
            nc.sync.dma_start(
                wk4_sb[:], wk4_d.rearrange("(eb p) m -> p eb m", p=128).bitcast(F32R)
            )
            wv_sb = wp.tile([128, 2, DK], F32R)
            nc.sync.dma_start(
                wv_sb[:], wv_d.rearrange("(eb p) m -> p eb m", p=128).bitcast(F32R)
            )
            wpf_sb = wp.tile([DK + 2, EMB + 2], F32R)
            nc.sync.dma_start(wpf_sb[:], wpf_d[:].bitcast(F32R))
            w1_sb = wp.tile([128, 2, HID], F32R)
            nc.sync.dma_start(
                w1_sb[:], w1_d.rearrange("(eb p) m -> p eb m", p=128).bitcast(F32R)
            )
            b1_sb = wp.tile([128, 8], F32)
            nc.sync.dma_start(b1_sb[:], b1_d[:])
            w2_sb = wp.tile([128, 8, EMB], F32R)
            nc.sync.dma_start(
                w2_sb[:], w2_d.rearrange("(hb p) m -> p hb m", p=128).bitcast(F32R)
            )
            if need_bcast or apply_b2:
                ones1_sb = wp.tile([1, 128], F32R)
                nc.vector.memset(ones1_sb[:].bitcast(I32), 0x3F800000)
                aff_sb = wp.tile([1, 5, EMB], F32R)
                nc.sync.dma_start(aff_sb[:], aff_d[:].bitcast(F32R))
            if need_bcast:
                with tc.tile_pool(name="psbc", bufs=1, space="PSUM") as psbc:
                    ps_b = psbc.tile([128, 4, EMB], F32, tag="bc")
                    for i in range(4):
                        nc.tensor.matmul(
                            ps_b[:, i], ones1_sb[:], aff_sb[:, 1 + i],
                            start=True, stop=True,
                        )
                    affb_sb = wp.tile([128, 4, EMB], F32)
                    nc.vector.tensor_copy(affb_sb[:], ps_b[:])

            def _emit_batches():
                for b in range(B_LOC):
                    # ---------- loads ----------
                    xt_sb = xq.tile([128, 2, T], F32R, tag="xt", bufs=2)
                    nc.sync.dma_start(
                        xt_sb[:],
                        xt_d[b].rearrange("(eb p) t -> p eb t", p=128).bitcast(F32R),
                    )
                    xp_sb = xq.tile([128, NT, EMB], F32, tag="xp", bufs=1)
                    nc.sync.dma_start(
                        xp_sb[:], xp_d[b].rearrange("(nt p) e -> p nt e", p=128)
                    )

                    if PH < 2:
                        nc.sync.dma_start(
                            out_d[b].rearrange("(nt p) e -> p nt e", p=128), xp_sb[:]
                        )
                        continue
                    qT_sb = qk_pool.tile([128, T], F32R, tag="qT", bufs=1)
                    kT_sb = qk_pool.tile([128, T], F32R, tag="kT", bufs=1)
                    v_ext = qk_pool.tile([128, NT, DK + 2], F32R, tag="v", bufs=1)
                    attn_sb = qk_pool.tile([DK + 2, T], F32R, tag="attn", bufs=1)

                    with tc.tile_pool(name="psatt", bufs=1, space="PSUM") as psatt:
                        # q,k projections (x4 replicated rows), per 512-chunk
                        for c in range(NCH):
                            for w4, dst in ((wq4_sb, qT_sb), (wk4_sb, kT_sb)):
                                ps_qk = psatt.tile(
                                    [128, 512], F32, tag="sc", bufs=4, name="ps_qk"
                                )
                                for eb in range(2):
                                    nc.tensor.matmul(
                                        ps_qk[:],
                                        w4[:, eb],
                                        xt_sb[:, eb, c * 512:(c + 1) * 512],
                                        start=(eb == 0), stop=(eb == 1),
                                    )
                                nc.scalar.copy(
                                    dst[:, c * 512:(c + 1) * 512], ps_qk[:]
                                )

                        # v projection (natural [s, dk]) + ones column
                        ps_v = psatt.tile([128, NT, DK], F32, tag="v", bufs=1)
                        for tb in range(NT):
                            for eb in range(2):
                                nc.tensor.matmul(
                                    ps_v[:, tb],
                                    xt_sb[:, eb, tb * 128:(tb + 1) * 128],
                                    wv_sb[:, eb],
                                    start=(eb == 0), stop=(eb == 1),
                                )
                        nc.vector.tensor_copy(v_ext[:, :, 0:DK], ps_v[:])
                        nc.vector.memset(v_ext[:, :, DK:DK + 1].bitcast(I32), 0x3F800000)
                        nc.vector.memset(v_ext[:, :, DK + 1:DK + 2].bitcast(I32), 0)

                        # attention: scoresT -> exp -> (diag mask) -> attn accum
                        for j in range(NCH):
                            t0 = j * 512
                            ps_at = psatt.tile([DK + 2, 512], F32, tag="at", bufs=2)
                            n_sb = 4 * j + 4
                            use_rt = not os.environ.get("KDBG_NO_RT")
                            for sb in range(n_sb):
                                lo = max(0, sb * 128 - t0)
                                grp = (sb % 4) * DK if use_rt else 0
                                ps_sc = psatt.tile([128, 512], F32, tag="sc", bufs=4)
                                nc.tensor.matmul(
                                    ps_sc[:, lo:512],
                                    kT_sb[grp:grp + DK, sb * 128:(sb + 1) * 128],
                                    qT_sb[grp:grp + DK, t0 + lo:t0 + 512],
                                    start=True, stop=True,
                                    tile_position=(grp, 0) if use_rt else None,
                                )
                                a_t = at_pool.tile([128, 512], F32R, tag="aT", bufs=4)
                                nc.scalar.activation(
                                    a_t[:, lo:512], ps_sc[:, lo:512], AF.Exp
                                )
                                if sb * 128 >= t0:  # diagonal block: causal mask
                                    nc.gpsimd.affine_select(
                                        out=a_t[:, lo:lo + 128],
                                        in_=a_t[:, lo:lo + 128],
                                        compare_op=OP.is_ge,
                                        fill=0.0,
                                        base=0,
                                        pattern=[[1, 128]],
                                        channel_multiplier=-1,
                                    )
                                nc.tensor.matmul(
                                    ps_at[:, lo:512],
                                    v_ext[:, sb, :],
                                    a_t[:, lo:512],
                                    start=(sb == 0), stop=(sb == n_sb - 1),
                                )
                            nc.vector.tensor_copy(attn_sb[:, t0:t0 + 512], ps_at[:])

                    if PH < 3:
                        nc.sync.dma_start(
                            out_d[b].rearrange("(nt p) e -> p nt e", p=128), xp_sb[:]
                        )
                        continue
                    # ---------- mh + z1 + LN1 + transpose ----------
                    z1_sb = big_pool.tile([128, NT, EMB], F32, tag="zres", bufs=1)
                    y1_sb = big_pool.tile([128, NT, EMB], F32, tag="y1", bufs=1)
                    recip = st_pool.tile([128, NT], F32, tag="recip", bufs=2)
                    y1T = [
                        big_pool.tile([128, T], F32R, tag=f"y1T{eb}", bufs=1, name=f"y1T{eb}")
                        for eb in range(2)
                    ]
                    with tc.tile_pool(name="psmh", bufs=1, space="PSUM") as psmh:
                        for tb in range(NT):
                            ps_mh = psmh.tile([128, EMB + 2], F32, tag="mh", bufs=4)
                            nc.tensor.matmul(
                                ps_mh[:],
                                attn_sb[:, tb * 128:(tb + 1) * 128],
                                wpf_sb[:],
                                start=True, stop=True,
                            )
                            nc.vector.reciprocal(
                                recip[:, tb:tb + 1], ps_mh[:, EMB:EMB + 1]
                            )
                            nc.vector.scalar_tensor_tensor(
                                out=z1_sb[:, tb],
                                in0=ps_mh[:, 0:EMB],
                                scalar=recip[:, tb:tb + 1],
                                in1=xp_sb[:, tb],
                                op0=OP.mult,
                                op1=OP.add,
                            )

                        _layernorm(nc, st_pool, sm_pool, z1_sb, y1_sb)
                        if apply_g1be1:
                            nc.vector.tensor_tensor(
                                y1_sb[:], y1_sb[:],
                                affb_sb[:, 0:1, :].to_broadcast([128, NT, EMB]),
                                OP.mult,
                            )
                            nc.vector.tensor_tensor(
                                y1_sb[:], y1_sb[:],
                                affb_sb[:, 1:2, :].to_broadcast([128, NT, EMB]),
                                OP.add,
                            )

                        for eb in range(2):
                            for half in range(2):
                                ps_tr = psmh.tile([128, 1024], F32, tag="tr", bufs=2)
                                for q in range(8):
                                    tb = half * 8 + q
                                    nc.tensor.transpose(
                                        ps_tr[:, q * 128:(q + 1) * 128],
                                        y1_sb[:, tb, eb * 128:(eb + 1) * 128],
                                        ident[:],
                                    )
                                nc.vector.tensor_copy(
                                    y1T[eb][:, half * 1024:(half + 1) * 1024], ps_tr[:]
                                )

                    if PH < 4:
                        nc.sync.dma_start(
                            out_d[b].rearrange("(nt p) e -> p nt e", p=128), y1_sb[:]
                        )
                        continue
                    # ---------- FFN + LN2 ----------
                    z2_sb = big_pool.tile([128, NT, EMB], F32, tag="zres", bufs=1)
                    y2_sb = big_pool.tile([128, NT, EMB], F32, tag="y2", bufs=1)
                    with tc.tile_pool(name="psffn", bufs=1, space="PSUM") as psffn:
                        for qtr in range(4):
                            hTg = [
                                h_pool.tile([128, 512], F32R, tag="hTg", bufs=8, name=f"hTg{_h}")
                                for _h in range(8)
                            ]
                            for h in range(8):
                                ps_h = psffn.tile([128, 512], F32, tag="h", bufs=2)
                                for eb in range(2):
                                    nc.tensor.matmul(
                                        ps_h[:],
                                        w1_sb[:, eb, h * 128:(h + 1) * 128],
                                        y1T[eb][:, qtr * 512:(qtr + 1) * 512],
                                        start=(eb == 0), stop=(eb == 1),
                                    )
                                nc.scalar.activation(
                                    hTg[h][:], ps_h[:], AF.Gelu, bias=b1_sb[:, h:h + 1]
                                )
                            ps_ff = psffn.tile([128, 4, EMB], F32, tag="ff", bufs=2)
                            for i in range(4):
                                if apply_b2:
                                    nc.tensor.matmul(
                                        ps_ff[:, i], ones1_sb[:], aff_sb[:, 0],
                                        start=True, stop=False,
                                    )
                                for h in range(8):
                                    tloc = i * 128
                                    nc.tensor.matmul(
                                        ps_ff[:, i],
                                        hTg[h][:, tloc:tloc + 128],
                                        w2_sb[:, h],
                                        start=(h == 0 and not apply_b2),
                                        stop=(h == 7),
                                    )
                            for i in range(4):
                                tb = qtr * 4 + i
                                nc.vector.scalar_tensor_tensor(
                                    out=z2_sb[:, tb],
                                    in0=ps_ff[:, i],
                                    scalar=1.0,
                                    in1=y1_sb[:, tb],
                                    op0=OP.mult,
                                    op1=OP.add,
                                )

                        _layernorm(nc, st_pool, sm_pool, z2_sb, y2_sb)
                        if apply_g2be2:
                            nc.vector.tensor_tensor(
                                y2_sb[:], y2_sb[:],
                                affb_sb[:, 2:3, :].to_broadcast([128, NT, EMB]),
                                OP.mult,
                            )
                            nc.vector.tensor_tensor(
                                y2_sb[:], y2_sb[:],
                                affb_sb[:, 3:4, :].to_broadcast([128, NT, EMB]),
                                OP.add,
                            )
                        nc.sync.dma_start(
                            out_d[b].rearrange("(nt p) e -> p nt e", p=128), y2_sb[:]
                        )

            LOOP_N = int(os.environ.get("KDBG_LOOP", "0"))
            if LOOP_N:
                with tc.For_i(0, LOOP_N, 1):
                    _emit_batches()
            else:
                _emit_batches()

    nc.compile()
    return nc


_CACHE = {}


def _get_nc(flags):
    if flags not in _CACHE:
        _CACHE[flags] = build_decoder(*flags)
    return _CACHE[flags]


def make_in_maps(x, Wq, Wk, Wv, Wp, bp, W1, b1, W2, b2, g1, be1, g2, be2):
    """Host-side preprocessing; returns per-core input maps + build flags."""
    f = np.asarray
    x = f(x, np.float32)
    wq4 = np.tile(f(Wq, np.float32) / math.sqrt(DK), (1, 4)).astype(np.float32)
    wk4 = np.tile(f(Wk, np.float32), (1, 4)).astype(np.float32)
    wpf = np.zeros((DK + 2, EMB + 2), np.float32)
    wpf[0:DK, 0:EMB] = f(Wp, np.float32).reshape(EMB // DK, DK, EMB).sum(axis=0)
    wpf[DK, EMB] = 1.0
    xp = (x + f(bp, np.float32)[None, None, :]).astype(np.float32)
    xt = np.ascontiguousarray(np.transpose(x, (0, 2, 1)))
    b1m = np.ascontiguousarray(f(b1, np.float32).reshape(8, 128).T)
    aff = np.stack(
        [f(b2), f(g1), f(be1), f(g2), f(be2)]
    ).astype(np.float32)[None]

    flags = (
        not (np.all(f(g1) == 1.0) and np.all(f(be1) == 0.0)),
        not (np.all(f(g2) == 1.0) and np.all(f(be2) == 0.0)),
        bool(np.any(f(b2) != 0.0)),
    )
    shared = {
        "wq4": wq4,
        "wk4": wk4,
        "wv": f(Wv, np.float32),
        "wpf": wpf,
        "w1": f(W1, np.float32),
        "b1": b1m,
        "w2": f(W2, np.float32),
        "aff": aff,
    }
    in_maps = []
    for c in range(N_CORES):
        sl = slice(c * B_LOC, (c + 1) * B_LOC)
        in_maps.append({"xp": xp[sl], "xt": xt[sl], **shared})
    return in_maps, flags


def kernel(**inputs) -> np.ndarray:
    in_maps, flags = make_in_maps(**inputs)
    nc = _get_nc(flags)
    res = run_bass_kernel_spmd(nc, in_maps, core_ids=list(range(N_CORES)))
    return np.concatenate([r["out"] for r in res.results], axis=0)



# revision 6
# speedup vs baseline: 1.1226x; 1.1226x over previous
"""Trainium2 Bass kernel for nn_Decoder_58377195487266.

Single-layer decoder: shared-head causal attention (d_k=32) + FFN(256->1024->256)
with two LayerNorms. B=16, T=2048, EMB=256.

Sharding: pure data-parallel over batch. 8 cores x 2 batches each, weights
replicated, no collectives.

v2: fp8e4 DoubleRow matmuls (2 stacked K-halves per PE pass) for the qkv
projections, AV, FFN1 and FFN2; bf16 for scores / output proj / transposes.
Host-side scale folding keeps fp8 operands out of the subnormal range:
  - wq,wk scaled x64 (wq also /sqrt(dk)); exp() applies scale 1/4096.
  - wv scaled x16; compensated by 1/16 inside the folded Wp.
  - w1,w2 scaled x64; gelu applies scale 1/64, z2-assembly multiplies 1/64.
Softmax denominator via ones-column in v + unit column in folded Wp
(normalization by reciprocal during z1 assembly) as in v1.
Engine split: ACT = exp/gelu only (paired into [128,2,512] instructions);
DVE = psum copies, z-assembly, LN stats; GPSIMD = causal masks, LN applies.
"""

import math
import os

import numpy as np

import concourse.bass as bass  # noqa: F401
import concourse.mybir as mybir
from concourse import bacc
from concourse.bass_utils import run_bass_kernel_spmd
from concourse.masks import make_identity
from concourse.tile import TileContext

F32 = mybir.dt.float32
BF16 = mybir.dt.bfloat16
F8 = mybir.dt.float8e4
I32 = mybir.dt.int32
AF = mybir.ActivationFunctionType
OP = mybir.AluOpType
DR = mybir.MatmulPerfMode.DoubleRow

B, T, EMB = 16, 2048, 256
DK = 32
HID = 4 * EMB
N_CORES = 8
B_LOC = B // N_CORES  # 2
NT = T // 128         # 16 t-blocks
NCH = T // 512        # 4 chunks
LN_EPS = 1e-5

SQK = 64.0   # q,k weight scale (folded); scores psum = 4096 * s
SV = 16.0    # v weight scale; compensated in wpf
SW = 64.0    # w1,w2 scale; compensated in gelu scale / z2 assembly

RSQRT_MAGIC = 0x5F3759DF


def _nr_rsqrt(nc, pool, out, varp):
    """out = 1/sqrt(varp) via bit-trick + 3 Newton iterations, all on DVE."""
    sh = list(varp.shape)
    yi = pool.tile(sh, I32, tag="nr_i", bufs=2)
    magic = pool.tile(sh, I32, tag="nr_m", bufs=2)
    nc.vector.memset(magic[:], RSQRT_MAGIC)
    nc.vector.tensor_scalar(yi[:], varp.bitcast(I32), 1, None, OP.logical_shift_right)
    nc.vector.tensor_tensor(yi[:], magic[:], yi[:], OP.subtract)
    y = yi.bitcast(F32)
    e = pool.tile(sh, F32, tag="nr_e", bufs=2)
    h = pool.tile(sh, F32, tag="nr_h", bufs=2)
    for _ in range(3):
        nc.vector.tensor_tensor(e[:], y[:], y[:], OP.mult)
        nc.vector.tensor_tensor(e[:], e[:], varp[:], OP.mult)
        nc.vector.tensor_scalar(h[:], e[:], -0.5, 1.5, OP.mult, OP.add)
        nc.vector.tensor_tensor(y[:], y[:], h[:], OP.mult)
    nc.vector.tensor_copy(out[:], y[:])


def _layernorm(nc, st_pool, z_sb, y_sb):
    """Stats on DVE (bn_stats), NR rsqrt, apply on GPSIMD. z/y any dtype."""
    stats6 = st_pool.tile([128, NT, 6], F32, tag="st6", bufs=2)
    agg = st_pool.tile([128, NT, 2], F32, tag="agg", bufs=2)
    for tb in range(NT):
        nc.vector.bn_stats(stats6[:, tb], z_sb[:, tb])
        nc.vector.bn_aggr(agg[:, tb], stats6[:, tb])
    mean = agg[:, :, 0]
    varp = st_pool.tile([128, NT], F32, tag="varp", bufs=2)
    rstd = st_pool.tile([128, NT], F32, tag="rstd", bufs=2)
    mrstd = st_pool.tile([128, NT], F32, tag="mrstd", bufs=2)
    nc.vector.tensor_scalar(varp[:], agg[:, :, 1], 1.0, LN_EPS, OP.mult, OP.add)
    _nr_rsqrt(nc, st_pool, rstd, varp)
    nc.vector.tensor_tensor(mrstd[:], mean, rstd[:], OP.mult)
    for tb in range(NT):
        nc.gpsimd.tensor_scalar(
            y_sb[:, tb], z_sb[:, tb],
            rstd[:, tb:tb + 1], mrstd[:, tb:tb + 1],
            OP.mult, OP.subtract,
        )


def build_decoder(apply_g1be1: bool, apply_g2be2: bool, apply_b2: bool,
                  apply_b1: bool):
    """Build the per-core Bass program (B_LOC batches, full T each)."""
    PH = int(os.environ.get("KDBG_PHASE", "99"))
    nc = bacc.Bacc(None, target_bir_lowering=False)

    xp_d = nc.dram_tensor("xp", [B_LOC, T, EMB], F32, kind="ExternalInput")
    xt8_d = nc.dram_tensor("xt8", [B_LOC, 128, 2, T], F8, kind="ExternalInput")
    wq8_d = nc.dram_tensor("wq8", [128, 2, 128], F8, kind="ExternalInput")
    wk8_d = nc.dram_tensor("wk8", [128, 2, 128], F8, kind="ExternalInput")
    wv8_d = nc.dram_tensor("wv8", [128, 2, DK], F8, kind="ExternalInput")
    wpf_d = nc.dram_tensor("wpf", [DK + 2, EMB + 2], BF16, kind="ExternalInput")
    w18_d = nc.dram_tensor("w18", [128, 2, HID], F8, kind="ExternalInput")
    b1_d = nc.dram_tensor("b1", [128, 8], F32, kind="ExternalInput")
    w28_d = nc.dram_tensor("w28", [128, 4, 2, EMB], F8, kind="ExternalInput")
    aff_d = nc.dram_tensor("aff", [1, 5, EMB], F32, kind="ExternalInput")
    # aff rows: b2, g1, be1, g2, be2
    out_d = nc.dram_tensor("out", [B_LOC, T, EMB], F32, kind="ExternalOutput")

    need_bcast = apply_g1be1 or apply_g2be2 or apply_b2

    with TileContext(nc) as tc:
        with (
            tc.tile_pool(name="wpool", bufs=1) as wp,
            tc.tile_pool(name="xpool", bufs=2) as xq,
            tc.tile_pool(name="qkpool", bufs=2) as qk_pool,
            tc.tile_pool(name="atpool", bufs=3) as at_pool,
            tc.tile_pool(name="bigpool", bufs=2) as big_pool,
            tc.tile_pool(name="hpool", bufs=5) as h_pool,
            tc.tile_pool(name="stats", bufs=2) as st_pool,
        ):
            # ---------- weights / constants ----------
            ident = wp.tile([128, 128], BF16)
            make_identity(nc, ident[:])
            wq8_sb = wp.tile([128, 2, 128], F8)
            nc.sync.dma_start(wq8_sb[:], wq8_d[:])
            wk8_sb = wp.tile([128, 2, 128], F8)
            nc.sync.dma_start(wk8_sb[:], wk8_d[:])
            wv8_sb = wp.tile([128, 2, DK], F8)
            nc.sync.dma_start(wv8_sb[:], wv8_d[:])
            wpf_sb = wp.tile([DK + 2, EMB + 2], BF16)
            nc.sync.dma_start(wpf_sb[:], wpf_d[:])
            w18_sb = wp.tile([128, 2, HID], F8)
            nc.sync.dma_start(w18_sb[:], w18_d[:])
            b1_sb = wp.tile([128, 8], F32)
            nc.sync.dma_start(b1_sb[:], b1_d[:])
            w28_sb = wp.tile([128, 4, 2, EMB], F8)
            nc.sync.dma_start(w28_sb[:], w28_d[:])
            if need_bcast:
                ones1_sb = wp.tile([1, 128], BF16)
                nc.vector.memset(ones1_sb[:], 1.0)
                aff_sb = wp.tile([1, 5, EMB], BF16)
                nc.sync.dma_start(aff_sb[:], aff_d[:])
                with tc.tile_pool(name="psbc", bufs=1, space="PSUM") as psbc:
                    ps_b = psbc.tile([128, 5, EMB], F32, tag="bc")
                    for i in range(5):
                        nc.tensor.matmul(
                            ps_b[:, i], ones1_sb[:], aff_sb[:, i],
                            start=True, stop=True,
                        )
                    affb_sb = wp.tile([128, 5, EMB], F32)
                    nc.vector.tensor_copy(affb_sb[:], ps_b[:])

            def _emit_batches():
                for b in range(B_LOC):
                    # ---------- loads ----------
                    xt8_sb = xq.tile([128, 2, T], F8, tag="xt", bufs=2)
                    nc.sync.dma_start(xt8_sb[:], xt8_d[b])
                    xp_sb = xq.tile([128, NT, EMB], F32, tag="xp", bufs=2)
                    nc.sync.dma_start(
                        xp_sb[:], xp_d[b].rearrange("(nt p) e -> p nt e", p=128)
                    )

                    if PH < 2:
                        nc.sync.dma_start(
                            out_d[b].rearrange("(nt p) e -> p nt e", p=128), xp_sb[:]
                        )
                        continue

                    qkT_sb = qk_pool.tile([128, 2, T], BF16, tag="qkT", bufs=2)
                    # last dim padded 34->48: DoubleRow lhsT outer free stride
                    # must be a multiple of 16 bytes (s3_lw_dual_fp8_restrictions)
                    v_ext = qk_pool.tile([128, NT // 2, 2, 48], F8, tag="v", bufs=2)
                    attn_sb = qk_pool.tile([DK + 2, T], BF16, tag="attn", bufs=2)

                    with tc.tile_pool(name="psatt", bufs=1, space="PSUM") as psatt:
                        # q,k projections (4 replicated col groups), per 512-chunk
                        for c in range(NCH):
                            ps_qk = psatt.tile([128, 2, 512], F32, tag="sc", bufs=2)
                            for i, w8 in ((0, wq8_sb), (1, wk8_sb)):
                                nc.tensor.matmul(
                                    ps_qk[:, i],
                                    w8[:],
                                    xt8_sb[:, :, c * 512:(c + 1) * 512],
                                    start=True, stop=True,
                                    perf_mode=DR,
                                )
                            nc.vector.tensor_copy(
                                qkT_sb[:, :, c * 512:(c + 1) * 512], ps_qk[:]
                            )

                        # v projection (natural [s, dk]) + ones column
                        ps_v = psatt.tile([128, NT // 2, 2, DK], F32, tag="v", bufs=1)
                        for tb in range(NT):
                            nc.tensor.matmul(
                                ps_v[:, tb // 2, tb % 2],
                                xt8_sb[:, :, tb * 128:(tb + 1) * 128],
                                wv8_sb[:],
                                start=True, stop=True,
                                perf_mode=DR,
                            )
                        nc.vector.tensor_copy(v_ext[:, :, :, 0:DK], ps_v[:])
                        nc.vector.memset(v_ext[:, :, :, DK:DK + 1], 1.0)
                        nc.vector.memset(v_ext[:, :, :, DK + 1:DK + 2], 0.0)

                        # attention: scoresT (bf16) -> exp(fp8) -> mask -> AV (fp8 DR)
                        for j in range(NCH):
                            t0 = j * 512
                            npair = 2 * j + 2
                            ps_at = psatt.tile([DK + 2, 512], F32, tag="at", bufs=2)
                            for p in range(npair):
                                sb0, sb1 = 2 * p, 2 * p + 1
                                lo0 = max(0, sb0 * 128 - t0)
                                lo1 = max(0, sb1 * 128 - t0)
                                ps_sc = psatt.tile([128, 2, 512], F32, tag="sc", bufs=2)
                                for i, sb in ((0, sb0), (1, sb1)):
                                    grp = (sb % 4) * DK
                                    nc.tensor.matmul(
                                        ps_sc[:, i, lo0:512],
                                        qkT_sb[grp:grp + DK, 1, sb * 128:(sb + 1) * 128],
                                        qkT_sb[grp:grp + DK, 0, t0 + lo0:t0 + 512],
                                        start=True, stop=True,
                                        tile_position=(grp, 0),
                                    )
                                a_t = at_pool.tile([128, 2, 512], F8, tag="aT", bufs=3)
                                nc.scalar.activation(
                                    a_t[:, :, lo0:512], ps_sc[:, :, lo0:512],
                                    AF.Exp, scale=1.0 / (SQK * SQK),
                                )
                                if sb0 * 128 >= t0:  # even diagonal block
                                    nc.gpsimd.affine_select(
                                        out=a_t[:, 0, lo0:lo0 + 128],
                                        in_=a_t[:, 0, lo0:lo0 + 128],
                                        compare_op=OP.is_ge,
                                        fill=0.0, base=0,
                                        pattern=[[1, 128]],
                                        channel_multiplier=-1,
                                    )
                                if sb1 * 128 >= t0:  # odd diagonal block
                                    nc.gpsimd.affine_select(
                                        out=a_t[:, 1, lo1:lo1 + 128],
                                        in_=a_t[:, 1, lo1:lo1 + 128],
                                        compare_op=OP.is_ge,
                                        fill=0.0, base=0,
                                        pattern=[[1, 128]],
                                        channel_multiplier=-1,
                                    )
                                    if lo1 > lo0:  # zero the below-block gap
                                        nc.gpsimd.memset(a_t[:, 1, lo0:lo1], 0.0)
                                nc.tensor.matmul(
                                    ps_at[:, lo0:512],
                                    v_ext[:, p, :, 0:DK + 2],
                                    a_t[:, :, lo0:512],
                                    start=(p == 0), stop=(p == npair - 1),
                                    perf_mode=DR,
                                )
                            nc.vector.tensor_copy(attn_sb[:, t0:t0 + 512], ps_at[:])

                    if PH < 3:
                        nc.sync.dma_start(
                            out_d[b].rearrange("(nt p) e -> p nt e", p=128), xp_sb[:]
                        )
                        continue
                    # ---------- mh + z1 + LN1 + transpose ----------
                    z1_sb = big_pool.tile([128, NT, EMB], BF16, tag="z1", bufs=2)
                    y1_sb = big_pool.tile([128, NT, EMB], BF16, tag="y1", bufs=2)
                    recip = st_pool.tile([128, NT], F32, tag="recip", bufs=2)
                    y1T_sb = big_pool.tile([128, 2, T], F8, tag="y1T", bufs=2)
                    with tc.tile_pool(name="psmh", bufs=1, space="PSUM") as psmh:
                        for tb in range(NT):
                            ps_mh = psmh.tile([128, EMB + 2], F32, tag="mh", bufs=4)
                            nc.tensor.matmul(
                                ps_mh[:],
                                attn_sb[:, tb * 128:(tb + 1) * 128],
                                wpf_sb[:],
                                start=True, stop=True,
                            )
                            nc.vector.reciprocal(
                                recip[:, tb:tb + 1], ps_mh[:, EMB:EMB + 1]
                            )
                            nc.vector.scalar_tensor_tensor(
                                out=z1_sb[:, tb],
                                in0=ps_mh[:, 0:EMB],
                                scalar=recip[:, tb:tb + 1],
                                in1=xp_sb[:, tb],
                                op0=OP.mult,
                                op1=OP.add,
                            )

                        _layernorm(nc, st_pool, z1_sb, y1_sb)
                        if apply_g1be1:
                            nc.vector.tensor_tensor(
                                y1_sb[:], y1_sb[:],
                                affb_sb[:, 1:2, :].to_broadcast([128, NT, EMB]),
                                OP.mult,
                            )
                            nc.vector.tensor_tensor(
                                y1_sb[:], y1_sb[:],
                                affb_sb[:, 2:3, :].to_broadcast([128, NT, EMB]),
                                OP.add,
                            )

                        for eb in range(2):
                            for half in range(2):
                                ps_tr = psmh.tile([128, 1024], BF16, tag="tr", bufs=2)
                                for q in range(8):
                                    tb = half * 8 + q
                                    nc.tensor.transpose(
                                        ps_tr[:, q * 128:(q + 1) * 128],
                                        y1_sb[:, tb, eb * 128:(eb + 1) * 128],
                                        ident[:],
                                    )
                                nc.vector.tensor_copy(
                                    y1T_sb[:, eb, half * 1024:(half + 1) * 1024],
                                    ps_tr[:],
                                )

                    if PH < 4:
                        nc.sync.dma_start(
                            out_d[b].rearrange("(nt p) e -> p nt e", p=128), xp_sb[:]
                        )
                        continue
                    # ---------- FFN (fp8 DR) + LN2 ----------
                    z2_sb = big_pool.tile([128, NT, EMB], BF16, tag="z2", bufs=2)
                    y2_sb = big_pool.tile([128, NT, EMB], F32, tag="y2", bufs=2)
                    with tc.tile_pool(name="psffn", bufs=1, space="PSUM") as psffn:
                        for qtr in range(4):
                            hTg = [
                                h_pool.tile([128, 2, 512], F8, tag="hTg", bufs=5,
                                            name=f"hTg{_h}")
                                for _h in range(4)
                            ]
                            for hp in range(4):
                                ps_h = psffn.tile([128, 2, 512], F32, tag="h", bufs=2)
                                for i in range(2):
                                    hb = 2 * hp + i
                                    nc.tensor.matmul(
                                        ps_h[:, i],
                                        w18_sb[:, :, hb * 128:(hb + 1) * 128],
                                        y1T_sb[:, :, qtr * 512:(qtr + 1) * 512],
                                        start=True, stop=True,
                                        perf_mode=DR,
                                    )
                                if apply_b1:
                                    for i in range(2):
                                        nc.scalar.activation(
                                            hTg[hp][:, i], ps_h[:, i], AF.Gelu,
                                            bias=b1_sb[:, 2 * hp + i:2 * hp + i + 1],
                                            scale=1.0 / SW,
                                        )
                                else:
                                    nc.scalar.activation(
                                        hTg[hp][:], ps_h[:], AF.Gelu, scale=1.0 / SW
                                    )
                            ps_ff = psffn.tile([128, 4, EMB], F32, tag="ff", bufs=2)
                            for i in range(4):
                                for hp in range(4):
                                    nc.tensor.matmul(
                                        ps_ff[:, i],
                                        hTg[hp][:, :, i * 128:(i + 1) * 128],
                                        w28_sb[:, hp],
                                        start=(hp == 0), stop=(hp == 3),
                                        perf_mode=DR,
                                    )
                            for i in range(4):
                                tb = qtr * 4 + i
                                nc.vector.scalar_tensor_tensor(
                                    out=z2_sb[:, tb],
                                    in0=ps_ff[:, i],
                                    scalar=1.0 / SW,
                                    in1=y1_sb[:, tb],
                                    op0=OP.mult,
                                    op1=OP.add,
                                )
                        if apply_b2:
                            nc.vector.tensor_tensor(
                                z2_sb[:], z2_sb[:],
                                affb_sb[:, 0:1, :].to_broadcast([128, NT, EMB]),
                                OP.add,
                            )

                        _layernorm(nc, st_pool, z2_sb, y2_sb)
                        if apply_g2be2:
                            nc.vector.tensor_tensor(
                                y2_sb[:], y2_sb[:],
                                affb_sb[:, 3:4, :].to_broadcast([128, NT, EMB]),
                                OP.mult,
                            )
                            nc.vector.tensor_tensor(
                                y2_sb[:], y2_sb[:],
                                affb_sb[:, 4:5, :].to_broadcast([128, NT, EMB]),
                                OP.add,
                            )
                        nc.sync.dma_start(
                            out_d[b].rearrange("(nt p) e -> p nt e", p=128), y2_sb[:]
                        )

            LOOP_N = int(os.environ.get("KDBG_LOOP", "0"))
            if LOOP_N:
                with tc.For_i(0, LOOP_N, 1):
                    _emit_batches()
            else:
                _emit_batches()

    nc.compile()
    return nc


_CACHE = {}


def _get_nc(flags):
    if flags not in _CACHE:
        _CACHE[flags] = build_decoder(*flags)
    return _CACHE[flags]


def _f8(a):
    f8np = mybir.dt.np(F8)
    return np.clip(np.asarray(a, np.float32), -240.0, 240.0).astype(f8np)


def _bf16(a):
    return np.asarray(a, np.float32).astype(mybir.dt.np(BF16))


def make_in_maps(x, Wq, Wk, Wv, Wp, bp, W1, b1, W2, b2, g1, be1, g2, be2):
    """Host-side preprocessing; returns per-core input maps + build flags."""
    f = np.asarray
    x = f(x, np.float32)
    # q/k weights: 4 replicated col groups, x64 scale (q also /sqrt(dk)),
    # [EMB, 128] -> pair layout [128, 2, 128]
    wq = np.tile(f(Wq, np.float32) * (SQK / math.sqrt(DK)), (1, 4))
    wk = np.tile(f(Wk, np.float32) * SQK, (1, 4))
    wq8 = _f8(wq.reshape(2, 128, 128).transpose(1, 0, 2))
    wk8 = _f8(wk.reshape(2, 128, 128).transpose(1, 0, 2))
    wv8 = _f8((f(Wv, np.float32) * SV).reshape(2, 128, DK).transpose(1, 0, 2))
    wpf = np.zeros((DK + 2, EMB + 2), np.float32)
    wpf[0:DK, 0:EMB] = f(Wp, np.float32).reshape(EMB // DK, DK, EMB).sum(axis=0) / SV
    wpf[DK, EMB] = 1.0
    xp = (x + f(bp, np.float32)[None, None, :]).astype(np.float32)
    xt8 = _f8(x.transpose(0, 2, 1).reshape(B, 2, 128, T).transpose(0, 2, 1, 3))
    w18 = _f8((f(W1, np.float32) * SW).reshape(2, 128, HID).transpose(1, 0, 2))
    w28 = _f8(
        (f(W2, np.float32) * SW).reshape(4, 2, 128, EMB).transpose(2, 0, 1, 3)
    )
    b1m = np.ascontiguousarray(f(b1, np.float32).reshape(8, 128).T)
    aff = np.stack(
        [f(b2), f(g1), f(be1), f(g2), f(be2)]
    ).astype(np.float32)[None]

    flags = (
        not (np.all(f(g1) == 1.0) and np.all(f(be1) == 0.0)),
        not (np.all(f(g2) == 1.0) and np.all(f(be2) == 0.0)),
        bool(np.any(f(b2) != 0.0)),
        bool(np.any(f(b1) != 0.0)),
    )
    shared = {
        "wq8": wq8,
        "wk8": wk8,
        "wv8": wv8,
        "wpf": _bf16(wpf),
        "w18": w18,
        "b1": b1m,
        "w28": w28,
        "aff": aff,
    }
    in_maps = []
    for c in range(N_CORES):
        sl = slice(c * B_LOC, (c + 1) * B_LOC)
        in_maps.append({"xp": xp[sl], "xt8": xt8[sl], **shared})
    return in_maps, flags


def kernel(**inputs) -> np.ndarray:
    in_maps, flags = make_in_maps(**inputs)
    nc = _get_nc(flags)
    res = run_bass_kernel_spmd(nc, in_maps, core_ids=list(range(N_CORES)))
    return np.concatenate([r["out"] for r in res.results], axis=0)
